# revision 4
# baseline (speedup 1.0000x reference)
"""BLSTM Trainium2 kernel (8-core SPMD).

Strategy
--------
B=32, S=128, H=1024, V=32000.  8 cores, tensor-parallel:

* Recurrence: TP-8 over the hidden dim.  Core j owns h-dims
  [128j, 128j+128) and the matching gate columns of W_ih/W_hh (i/f/g/o
  each sliced to 128 cols -> 512 gate cols per core).  Each step the
  core computes its gate slice, updates (c,h) for its chunk, and an
  AllGather of the bf16 h-chunks gives every core the full h for the
  next step.  Both directions run interleaved so the AllGather latency
  of one direction hides behind compute of the other.
* Input projection xp = emb @ W_ih + b is precomputed per 128-row
  block (row = (s, b), s-major), spread through the sequence so block
  j lands just before steps 4j..4j+4 (fwd ascending, bwd descending).
  Embedding rows are gathered with indirect DMA, transposed on the PE.
* fc_out: vocab-sharded (4000 cols/core).  lhsT = combined^T row-block
  tiles (from the per-step AllGather outputs in DRAM), moving = W_out
  (bf16, SBUF-resident in two 2000-col halves).  Emitted interleaved
  with the recurrence as soon as row-blocks are ready so it fills PE
  idle time during AllGather waits.

Outputs: logits [B,S,V] (f32, concat of per-core vocab shards) and the
final states (hf, cf, hb, cb) [B,H] f32.
"""

import sys

for _p in ("/opt/trn_rl_repo",):
    if _p not in sys.path:
        sys.path.insert(0, _p)

import numpy as np

import concourse.bass as bass
import concourse.bacc as bacc
import concourse.mybir as mybir
from concourse import tile
from concourse.bass import IndirectOffsetOnAxis
from concourse.bass_utils import run_bass_kernel_spmd

F32 = mybir.dt.float32
BF16 = mybir.dt.bfloat16
I32DT = mybir.dt.int32
AF = mybir.ActivationFunctionType

N_CORES = 8
B = 32
H = 1024
HC = H // N_CORES          # 128, per-core h-chunk
KT = H // 128              # 8 k-tiles over H
G = 4 * HC                 # 512 gate cols per core


def build_nc(S=128, VSH=4000, VEMB=32000, n_cores=N_CORES):
    """Emit the BIR program (same program on every core)."""
    assert S % 4 == 0
    NRB = S // 4               # 128-row blocks over (s,b) rows
    VHALF = VSH // 2
    assert VSH % 2 == 0

    # n-chunks (<=500 cols, 4 psum banks) within one W_out half
    NCH = []
    off = 0
    while off < VHALF:
        n = min(500, VHALF - off)
        NCH.append((off, n))
        off += n

    nc = bacc.Bacc(
        "TRN2",
        target_bir_lowering=False,
        debug=False,
        num_devices=n_cores,
    )
    rg = [list(range(n_cores))]

    # ---------------- I/O ----------------
    x_d = nc.dram_tensor("x", [B, S], I32DT, kind="ExternalInput")
    emb_d = nc.dram_tensor("embed_bf", [VEMB, H], BF16, kind="ExternalInput")
    wih_d = {d: nc.dram_tensor(f"wih_{d}", [128, KT, G], BF16, kind="ExternalInput")
             for d in "fb"}
    whh_d = {d: nc.dram_tensor(f"whh_{d}", [128, KT, 4, HC], BF16, kind="ExternalInput")
             for d in "fb"}
    bsum_d = {d: nc.dram_tensor(f"bsum_{d}", [1, G], BF16, kind="ExternalInput")
              for d in "fb"}
    wout_d = nc.dram_tensor("wout", [128, 2 * KT, VSH], BF16, kind="ExternalInput")
    bout_d = nc.dram_tensor("bout", [1, VSH], BF16, kind="ExternalInput")
    i32f_d = nc.dram_tensor("i32f", [32, 32], F32, kind="ExternalInput")
    i128b_d = nc.dram_tensor("i128b", [128, 128], BF16, kind="ExternalInput")
    i128f_d = nc.dram_tensor("i128f", [128, 128], F32, kind="ExternalInput")
    ones_d = nc.dram_tensor("onesb", [1, 128], BF16, kind="ExternalInput")

    logits_d = nc.dram_tensor("logits", [B, S, VSH], F32, kind="ExternalOutput")
    st_out_d = {k: nc.dram_tensor(k, [B, H], F32, kind="ExternalOutput")
                for k in ("hf", "cf", "hb", "cb")}

    DIRS = ("f", "b")

    with tile.TileContext(nc) as tc:
        sb = tc.alloc_tile_pool(name="sb", bufs=1, space="SBUF")
        ps = tc.alloc_tile_pool(name="ps", bufs=1, space="PSUM")
        dr = tc.alloc_tile_pool(name="dr", bufs=1, space="DRAM")

        # ---------------- constants + weights into SBUF ----------------
        x_sb = sb.tile([B, S], I32DT, tag="x")
        nc.sync.dma_start(x_sb[:], x_d[:, :])
        i32f = sb.tile([32, 32], F32, tag="i32f")
        nc.sync.dma_start(i32f[:], i32f_d[:, :])
        i128b = sb.tile([128, 128], BF16, tag="i128b")
        nc.sync.dma_start(i128b[:], i128b_d[:, :])
        i128f = sb.tile([128, 128], F32, tag="i128f")
        nc.sync.dma_start(i128f[:], i128f_d[:, :])
        ones = sb.tile([1, 128], BF16, tag="ones")
        nc.sync.dma_start(ones[:], ones_d[:, :])
        bout_sb = sb.tile([1, VSH], BF16, tag="bout")
        nc.sync.dma_start(bout_sb[:], bout_d[:, :])

        wih = {}
        whh = {}
        bsum = {}
        for d in DIRS:
            wih[d] = sb.tile([128, KT, G], BF16, tag=f"wih{d}", name=f"wih{d}")
            nc.sync.dma_start(wih[d][:], wih_d[d][:, :, :])
            whh[d] = sb.tile([128, KT, 4, HC], BF16, tag=f"whh{d}", name=f"whh{d}")
            nc.sync.dma_start(whh[d][:], whh_d[d][:, :, :, :])
            bsum[d] = sb.tile([1, G], BF16, tag=f"bs{d}", name=f"bs{d}")
            nc.sync.dma_start(bsum[d][:], bsum_d[d][:, :])

        # W_out half currently resident (two passes over rbs)
        wout_sb = [None]

        def load_wout_half(h):
            t = sb.tile([128, 2 * KT, VHALF], BF16, tag="wout", name=f"wout_h{h}")
            nc.sync.dma_start(t[:], wout_d[:, :, h * VHALF:(h + 1) * VHALF])
            wout_sb[0] = t

        load_wout_half(0)

        # ---------------- psum layout ----------------
        # gates_f 1 + gates_b 1 + tps 1 + xp 1 + fc 4 = 8 banks
        gates_ps = {d: ps.tile([128, 256], F32, tag=f"g{d}", name=f"gates_{d}")
                    for d in DIRS}
        tps = ps.tile([128, 512], BF16, tag="tps")
        xp_ps = ps.tile([128, 512], F32, tag="xpps")
        fc_ps = ps.tile([128, 2048], F32, tag="fcps")

        # ---------------- prologue-spread input projection ----------------
        xp_dram = {d: [None] * NRB for d in DIRS}
        tp_slot = [0]

        def emit_xp_rb(d, rb):
            # rows rb*128 .. rb*128+128  (s = 4rb..4rb+4, b inner)
            emb_sb = sb.tile([128, H], BF16, tag="embg", bufs=2, name=f"emb_{d}{rb}")
            for si in range(4):
                s = 4 * rb + si
                nc.gpsimd.indirect_dma_start(
                    emb_sb[32 * si:32 * (si + 1), :],
                    None,
                    emb_d[:, :],
                    IndirectOffsetOnAxis(ap=x_sb[:, s:s + 1], axis=0),
                )
            embT = sb.tile([128, KT, 128], BF16, tag="embT", bufs=2, name=f"embT_{d}{rb}")
            for k in range(KT):
                slot = tp_slot[0] % 4
                tp_slot[0] += 1
                nc.tensor.transpose(
                    tps[:, 128 * slot:128 * (slot + 1)],
                    emb_sb[:, 128 * k:128 * (k + 1)],
                    i128b[:, :],
                )
                nc.vector.tensor_copy(embT[:, k, :], tps[:, 128 * slot:128 * (slot + 1)])
            for k in range(KT):
                nc.tensor.matmul(
                    xp_ps[:, 0:G], embT[:, k, :], wih[d][:, k, :],
                    start=(k == 0), stop=False,
                )
            nc.tensor.matmul(xp_ps[:, 0:G], ones[:, :], bsum[d][:, :],
                             start=False, stop=True)
            xp_out = sb.tile([128, G], F32, tag="xpout", bufs=2, name=f"xpo_{d}{rb}")
            nc.scalar.copy(xp_out[:], xp_ps[:, 0:G])
            xpd = dr.tile([128, G], F32, tag=f"xp{d}{rb}", name=f"xpd_{d}{rb}")
            nc.sync.dma_start(xpd[:], xp_out[:])
            xp_dram[d][rb] = xpd

        # ---------------- recurrence ----------------
        c_prev = {d: None for d in DIRS}
        h_all = {d: None for d in DIRS}      # [128, KT, 32] bf16 (gathered h)
        hst = {d: [None] * S for d in DIRS}  # per-step AG outputs (DRAM)
        h_last_f32 = {}
        c_last = {}

        def emit_step(d, t):
            """dir-step: fwd processes s=t, bwd processes s=S-1-t."""
            s = t if d == "f" else S - 1 - t
            buf = t % 2
            gsl = gates_ps[d][:, 128 * buf:128 * (buf + 1)]

            xp_sb = sb.tile([32, G], F32, tag=f"xps{d}", bufs=3, name=f"xp_{d}{t}")
            rb, si = s // 4, s % 4
            nc.sync.dma_start(xp_sb[:], xp_dram[d][rb][32 * si:32 * (si + 1), :])

            # seed psum with xp (start=True clears the bank's has_written)
            for m in range(4):
                nc.tensor.matmul(
                    gsl[:, 32 * m:32 * (m + 1)],
                    xp_sb[:, 128 * m:128 * (m + 1)], i32f[:, :],
                    start=(m == 0), stop=(m == 3 and t == 0),
                )
            if t > 0:
                ha = h_all[d]
                for m in range(4):
                    for k in range(KT):
                        nc.tensor.matmul(
                            gsl[:, 32 * m:32 * (m + 1)],
                            whh[d][:, k, m, :], ha[:, k, :],
                            start=False,
                            stop=(m == 3 and k == KT - 1),
                        )

            # activations + cell update
            act = {}
            for m, (nm, fn) in enumerate(
                    (("i", AF.Sigmoid), ("f", AF.Sigmoid),
                     ("g", AF.Tanh), ("o", AF.Sigmoid))):
                a = sb.tile([128, 32], F32, tag=f"a{nm}{d}", bufs=2, name=f"{nm}_{d}{t}")
                nc.scalar.activation(a[:], gsl[:, 32 * m:32 * (m + 1)], fn)
                act[nm] = a

            ig = sb.tile([128, 32], F32, tag=f"ig{d}", bufs=2, name=f"ig_{d}{t}")
            nc.vector.tensor_mul(ig[:], act["i"][:], act["g"][:])
            c_new = sb.tile([128, 32], F32, tag=f"c{d}", bufs=2, name=f"c_{d}{t}")
            if t == 0:
                nc.vector.tensor_copy(c_new[:], ig[:])
            else:
                fc = sb.tile([128, 32], F32, tag=f"fcm{d}", bufs=2, name=f"fcm_{d}{t}")
                nc.vector.tensor_mul(fc[:], act["f"][:], c_prev[d][:])
                nc.vector.tensor_add(c_new[:], fc[:], ig[:])
            c_prev[d] = c_new
            tc_sb = sb.tile([128, 32], F32, tag=f"tc{d}", bufs=2, name=f"tc_{d}{t}")
            nc.scalar.activation(tc_sb[:], c_new[:], AF.Tanh)
            h_f32 = sb.tile([128, 32], F32, tag=f"hf32{d}", bufs=2, name=f"h32_{d}{t}")
            nc.vector.tensor_mul(h_f32[:], act["o"][:], tc_sb[:])
            h_bf = sb.tile([128, 32], BF16, tag=f"h{d}", bufs=2, name=f"h_{d}{t}")
            nc.vector.tensor_copy(h_bf[:], h_f32[:])

            if t == S - 1:
                h_last_f32[d] = h_f32
                c_last[d] = c_new

            # exchange h chunk
            agin = dr.tile([128, 32], BF16, tag=f"agin{d}", bufs=2, name=f"agin_{d}{t}")
            nc.sync.dma_start(agin[:], h_bf[:])
            hout = dr.tile([n_cores * 128, 32], BF16, tag=f"hst{d}{t}", name=f"hst_{d}{t}")
            nc.gpsimd.collective_compute(
                "AllGather", mybir.AluOpType.bypass,
                replica_groups=rg, ins=[agin.opt()], outs=[hout.opt()],
            )
            hst[d][s] = hout
            if t < S - 1:
                ha = sb.tile([128, KT, 32], BF16, tag=f"hall{d}", bufs=2,
                             name=f"hall_{d}{t}")
                nc.sync.dma_start(
                    ha[:], hout.rearrange("(k p) b -> p k b", p=128))
                h_all[d] = ha

        # ---------------- fc_out pieces ----------------
        def emit_fc_rb(rb, half):
            lhs = sb.tile([128, 2 * KT, 4, 32], BF16, tag="fclhs", bufs=2,
                          name=f"fclhs_{rb}h{half}")
            for di, d in enumerate(DIRS):
                for si in range(4):
                    s = 4 * rb + si
                    nc.sync.dma_start(
                        lhs[:, di * KT:(di + 1) * KT, si, :],
                        hst[d][s].rearrange("(k p) b -> p k b", p=128),
                    )
            vbase = half * VHALF
            for g0 in range(0, len(NCH), 4):
                grp = NCH[g0:g0 + 4]
                for k in range(2 * KT):
                    for gi, (noff, nsz) in enumerate(grp):
                        bank = (g0 + gi) % 4
                        nc.tensor.matmul(
                            fc_ps[:, 512 * bank:512 * bank + nsz],
                            lhs[:, k, :, :],
                            wout_sb[0][:, k, noff:noff + nsz],
                            start=(k == 0), stop=False,
                        )
                for gi, (noff, nsz) in enumerate(grp):
                    bank = (g0 + gi) % 4
                    nc.tensor.matmul(
                        fc_ps[:, 512 * bank:512 * bank + nsz],
                        ones[:, :], bout_sb[:, vbase + noff:vbase + noff + nsz],
                        start=False, stop=True,
                    )
                for gi, (noff, nsz) in enumerate(grp):
                    bank = (g0 + gi) % 4
                    o_sb = sb.tile([128, 500], F32, tag="fco", bufs=4,
                                   name=f"fco_{rb}_{half}_{g0 + gi}")
                    if gi % 2 == 0:
                        nc.scalar.copy(o_sb[:, 0:nsz],
                                       fc_ps[:, 512 * bank:512 * bank + nsz])
                    else:
                        nc.vector.tensor_copy(o_sb[:, 0:nsz],
                                              fc_ps[:, 512 * bank:512 * bank + nsz])
                    # psum rows are (si, b) s-major; logits is [B, S, VSH]
                    dst = logits_d[:, 4 * rb:4 * rb + 4,
                                   vbase + noff:vbase + noff + nsz]
                    dst = dst.transpose([1, 0, 2])   # (si, b, n) iteration
                    nc.sync.dma_start(dst, o_sb[:, 0:nsz])

        # ---------------- emission schedule ----------------
        emit_xp_rb("f", 0)
        emit_xp_rb("b", NRB - 1)
        emit_xp_rb("f", 1)
        emit_xp_rb("b", NRB - 2)

        fc_queue = []
        fc_ready = set()
        next_xp = 2

        for t in range(S):
            emit_step("f", t)
            emit_step("b", t)
            if t % 4 == 1 and next_xp < NRB:
                emit_xp_rb("f", next_xp)
                emit_xp_rb("b", NRB - 1 - next_xp)
                next_xp += 1
            for rb in range(NRB):
                if rb not in fc_ready and t >= max(4 * rb + 3, S - 1 - 4 * rb):
                    fc_ready.add(rb)
                    fc_queue.append((rb, 0))
            if fc_queue and t >= 2:
                rb, half = fc_queue.pop(0)
                emit_fc_rb(rb, half)

        while fc_queue:
            rb, half = fc_queue.pop(0)
            emit_fc_rb(rb, half)
        load_wout_half(1)
        for rb in range(NRB):
            emit_fc_rb(rb, 1)

        # ---------------- final states ----------------
        st_sb = sb.tile([128, 4, 32], F32, tag="stin")
        for i, src in enumerate((h_last_f32["f"], c_last["f"],
                                 h_last_f32["b"], c_last["b"])):
            nc.vector.tensor_copy(st_sb[:, i, :], src[:])
        st_in = dr.tile([128, 128], F32, tag="stagin")
        nc.sync.dma_start(st_in[:], st_sb[:])
        st_all = dr.tile([n_cores * 128, 128], F32, tag="stall")
        nc.gpsimd.collective_compute(
            "AllGather", mybir.AluOpType.bypass,
            replica_groups=rg, ins=[st_in.opt()], outs=[st_all.opt()],
        )
        st_all_sb = sb.tile([128, n_cores, 128], F32, tag="stallsb")
        nc.sync.dma_start(st_all_sb[:],
                          st_all.rearrange("(c p) q -> p c q", p=128))
        stps = gates_ps["f"]     # gates banks are free after the last step
        for i, nm in enumerate(("hf", "cf", "hb", "cb")):
            out_sb = sb.tile([32, H], F32, tag="stout", bufs=2, name=f"st_{nm}")
            for c in range(n_cores):
                nc.tensor.transpose(
                    stps[0:32, 0:128],
                    st_all_sb[:, c, 32 * i:32 * (i + 1)],
                    i128f[:, :],
                )
                nc.vector.tensor_copy(out_sb[:, 128 * c:128 * (c + 1)],
                                      stps[0:32, 0:128])
            nc.sync.dma_start(st_out_d[nm][:, :], out_sb[:])

        sb.release()
        ps.release()
        dr.release()

    nc.compile()
    return nc


# ---------------------------------------------------------------------------
# host side
# ---------------------------------------------------------------------------

def make_in_maps(inputs, VSH=4000, n_cores=N_CORES):
    import ml_dtypes
    bf = ml_dtypes.bfloat16
    x = np.asarray(inputs["x"], np.int32)
    embed = np.asarray(inputs["embed"], np.float32)
    W = {d: {k: np.asarray(inputs[f"{k}_{d}"], np.float32)
             for k in ("W_ih", "b_ih", "W_hh", "b_hh")} for d in "fb"}
    W_out = np.asarray(inputs["W_out"], np.float32)
    b_out = np.asarray(inputs["b_out"], np.float32)
    Hh = W["f"]["W_ih"].shape[0]
    kt = Hh // 128
    hc = Hh // n_cores
    embed_bf = embed.astype(bf)
    in_maps = []
    for j in range(n_cores):
        cols = np.concatenate(
            [np.arange(g * Hh + j * hc, g * Hh + (j + 1) * hc) for g in range(4)])
        m = {
            "x": x,
            "embed_bf": embed_bf,
            "i32f": np.eye(32, dtype=np.float32),
            "i128b": np.eye(128, dtype=np.float32).astype(bf),
            "i128f": np.eye(128, dtype=np.float32),
            "onesb": np.ones((1, 128), np.float32).astype(bf),
        }
        g4 = 4 * hc
        for d in "fb":
            wihp = W[d]["W_ih"][:, cols].reshape(kt, 128, g4).transpose(1, 0, 2)
            m[f"wih_{d}"] = np.ascontiguousarray(wihp).astype(bf)
            whhp = (W[d]["W_hh"][:, cols]
                    .reshape(kt, 128, 4, hc).transpose(1, 0, 2, 3))
            m[f"whh_{d}"] = np.ascontiguousarray(whhp).astype(bf)
            m[f"bsum_{d}"] = ((W[d]["b_ih"] + W[d]["b_hh"])[cols][None, :]).astype(bf)
        vs = j * VSH
        wo = W_out[:, vs:vs + VSH].reshape(2 * kt, 128, VSH).transpose(1, 0, 2)
        m["wout"] = np.ascontiguousarray(wo).astype(bf)
        m["bout"] = b_out[vs:vs + VSH][None, :].astype(bf)
        in_maps.append(m)
    return in_maps


_NC_CACHE = {}


def _get_nc(key=(128, 4000, 32000)):
    if key not in _NC_CACHE:
        _NC_CACHE[key] = build_nc(S=key[0], VSH=key[1], VEMB=key[2])
    return _NC_CACHE[key]


def kernel(**inputs):
    S, VSH = 128, 4000
    nc = _get_nc((S, VSH, 32000))
    in_maps = make_in_maps(inputs, VSH=VSH)
    res = run_bass_kernel_spmd(nc, in_maps, core_ids=list(range(N_CORES)))
    outs = res.results
    logits = np.concatenate(
        [np.asarray(outs[j]["logits"], np.float32) for j in range(N_CORES)], axis=-1)
    st = outs[0]
    return (logits,
            (np.asarray(st["hf"], np.float32), np.asarray(st["cf"], np.float32),
             np.asarray(st["hb"], np.float32), np.asarray(st["cb"], np.float32)))


# revision 21
# speedup vs baseline: 12.4838x; 12.4838x over previous
"""BLSTM Trainium2 kernel (8-core SPMD).

Strategy
--------
B=32, S=128, H=1024, V=32000.  8 cores, tensor-parallel:

* Recurrence: TP-8 over the hidden dim.  Core j owns h-dims
  [128j, 128j+128) and the matching gate columns of W_ih/W_hh (i/f/g/o
  each sliced to 128 cols -> 512 gate cols per core).  Each step the
  core computes its gate slice, updates (c,h) for its chunk, and an
  AllGather of the bf16 h-chunks gives every core the full h for the
  next step.  Both directions run interleaved so the AllGather latency
  of one direction hides behind compute of the other.
* Input projection xp = emb @ W_ih + b is precomputed per 128-row
  block (row = (s, b), s-major), spread through the sequence so block
  j lands just before steps 4j..4j+4 (fwd ascending, bwd descending).
  Embedding rows are gathered with indirect DMA, transposed on the PE.
* fc_out: vocab-sharded (4000 cols/core).  lhsT = combined^T row-block
  tiles (from the per-step AllGather outputs in DRAM), moving = W_out
  (bf16, SBUF-resident in two 2000-col halves).  Emitted interleaved
  with the recurrence as soon as row-blocks are ready so it fills PE
  idle time during AllGather waits.

Outputs: logits [B,S,V] (f32, concat of per-core vocab shards) and the
final states (hf, cf, hb, cb) [B,H] f32.
"""

import sys

for _p in ("/opt/trn_rl_repo",):
    if _p not in sys.path:
        sys.path.insert(0, _p)

import numpy as np

import concourse.bass as bass
import concourse.bacc as bacc
import concourse.mybir as mybir
from concourse import tile
from concourse.bass import IndirectOffsetOnAxis
from concourse.bass_utils import run_bass_kernel_spmd

F32 = mybir.dt.float32
BF16 = mybir.dt.bfloat16
I32DT = mybir.dt.int32
AF = mybir.ActivationFunctionType

N_CORES = 8
B = 32
H = 1024
HC = H // N_CORES          # 128, per-core h-chunk
KT = H // 128              # 8 k-tiles over H
G = 4 * HC                 # 512 gate cols per core


def build_nc(S=128, VSH=4000, VEMB=32000, n_cores=N_CORES):
    """Emit the BIR program (same program on every core)."""
    assert S % 4 == 0
    NRB = S // 4               # 128-row blocks over (s,b) rows
    VHALF = VSH // 2
    assert VSH % 2 == 0

    # n-chunks (<=500 cols, 4 psum banks) within one W_out half
    NCH = []
    off = 0
    while off < VHALF:
        n = min(500, VHALF - off)
        NCH.append((off, n))
        off += n

    nc = bacc.Bacc(
        "TRN2",
        target_bir_lowering=False,
        debug=False,
        num_devices=n_cores,
    )
    rg = [list(range(n_cores))]

    # ---------------- I/O ----------------
    x_d = nc.dram_tensor("x", [B, S], I32DT, kind="ExternalInput")
    emb_d = nc.dram_tensor("embed_bf", [VEMB, H], BF16, kind="ExternalInput")
    wih_d = {d: nc.dram_tensor(f"wih_{d}", [128, KT, G], BF16, kind="ExternalInput")
             for d in "fb"}
    whh_d = {d: nc.dram_tensor(f"whh_{d}", [128, KT, 4, HC], BF16, kind="ExternalInput")
             for d in "fb"}
    bsum_d = {d: nc.dram_tensor(f"bsum_{d}", [1, G], BF16, kind="ExternalInput")
              for d in "fb"}
    wout_d = nc.dram_tensor("wout", [128, 2 * KT, VSH], BF16, kind="ExternalInput")
    bout_d = nc.dram_tensor("bout", [1, VSH], BF16, kind="ExternalInput")
    i32f_d = nc.dram_tensor("i32f", [32, 32], F32, kind="ExternalInput")
    i128b_d = nc.dram_tensor("i128b", [128, 128], BF16, kind="ExternalInput")
    i128f_d = nc.dram_tensor("i128f", [128, 128], F32, kind="ExternalInput")
    ones_d = nc.dram_tensor("onesb", [1, 128], BF16, kind="ExternalInput")

    logits_d = nc.dram_tensor("logits", [B, S, VSH], F32, kind="ExternalOutput")
    st_out_d = {k: nc.dram_tensor(k, [B, H], F32, kind="ExternalOutput")
                for k in ("hf", "cf", "hb", "cb")}

    DIRS = ("f", "b")

    with tile.TileContext(nc) as tc:
        sb = tc.alloc_tile_pool(name="sb", bufs=1, space="SBUF")
        ps = tc.alloc_tile_pool(name="ps", bufs=1, space="PSUM")
        dr = tc.alloc_tile_pool(name="dr", bufs=1, space="DRAM")

        # ---------------- constants + weights into SBUF ----------------
        x_sb = sb.tile([B, S], I32DT, tag="x")
        nc.sync.dma_start(x_sb[:], x_d[:, :])
        i32f = sb.tile([32, 32], F32, tag="i32f")
        nc.sync.dma_start(i32f[:], i32f_d[:, :])
        i128b = sb.tile([128, 128], BF16, tag="i128b")
        nc.sync.dma_start(i128b[:], i128b_d[:, :])
        i128f = sb.tile([128, 128], F32, tag="i128f")
        nc.sync.dma_start(i128f[:], i128f_d[:, :])
        ones = sb.tile([1, 128], BF16, tag="ones")
        nc.sync.dma_start(ones[:], ones_d[:, :])
        bout_sb = sb.tile([1, VSH], BF16, tag="bout")
        nc.sync.dma_start(bout_sb[:], bout_d[:, :])

        wih = {}
        whh = {}
        bsum = {}
        for d in DIRS:
            wih[d] = sb.tile([128, KT, G], BF16, tag=f"wih{d}", name=f"wih{d}")
            nc.scalar.dma_start(wih[d][:], wih_d[d][:, :, :])
            whh[d] = sb.tile([128, KT, 4, HC], BF16, tag=f"whh{d}", name=f"whh{d}")
            nc.scalar.dma_start(whh[d][:], whh_d[d][:, :, :, :])
            bsum[d] = sb.tile([1, G], BF16, tag=f"bs{d}", name=f"bs{d}")
            nc.sync.dma_start(bsum[d][:], bsum_d[d][:, :])

        # one W_out half resident at a time (two passes over rbs)
        wout_sb = [None]

        def load_wout_half(h):
            t = sb.tile([128, 2 * KT, VHALF], BF16, tag="wout", name=f"wout_h{h}")
            nc.scalar.dma_start(t[:], wout_d[:, :, h * VHALF:(h + 1) * VHALF])
            wout_sb[0] = t

        load_wout_half(0)

        # ---------------- psum layout ----------------
        # gates_f 1 + gates_b 1 + tps 1 + xp 1 + fc 4 = 8 banks
        gates_ps = {d: ps.tile([128, 256], F32, tag=f"g{d}", name=f"gates_{d}")
                    for d in DIRS}
        tps = ps.tile([128, 512], BF16, tag="tps")
        xp_ps = ps.tile([128, 512], F32, tag="xpps")
        fc_ps = ps.tile([128, 2048], F32, tag="fcps")

        # ---------------- prologue-spread input projection ----------------
        xp_dram = {d: [None] * NRB for d in DIRS}
        tp_slot = [0]

        def emit_xp_rb(d, rb):
            # rows rb*128 .. rb*128+128  (s = 4rb..4rb+4, b inner)
            emb_sb = sb.tile([128, H], BF16, tag="embg", bufs=2, name=f"emb_{d}{rb}")
            for si in range(4):
                s = 4 * rb + si
                nc.gpsimd.indirect_dma_start(
                    emb_sb[32 * si:32 * (si + 1), :],
                    None,
                    emb_d[:, :],
                    IndirectOffsetOnAxis(ap=x_sb[:, s:s + 1], axis=0),
                )
            embT = sb.tile([128, KT, 128], BF16, tag="embT", bufs=2, name=f"embT_{d}{rb}")
            for k in range(KT):
                slot = tp_slot[0] % 4
                tp_slot[0] += 1
                nc.tensor.transpose(
                    tps[:, 128 * slot:128 * (slot + 1)],
                    emb_sb[:, 128 * k:128 * (k + 1)],
                    i128b[:, :],
                )
                nc.vector.tensor_copy(embT[:, k, :], tps[:, 128 * slot:128 * (slot + 1)])
            for k in range(KT):
                nc.tensor.matmul(
                    xp_ps[:, 0:G], embT[:, k, :], wih[d][:, k, :],
                    start=(k == 0), stop=False,
                )
            nc.tensor.matmul(xp_ps[:, 0:G], ones[:, :], bsum[d][:, :],
                             start=False, stop=True)
            xp_out = sb.tile([128, G], F32, tag="xpout", bufs=2, name=f"xpo_{d}{rb}")
            nc.scalar.copy(xp_out[:], xp_ps[:, 0:G])
            xpd = dr.tile([128, G], F32, tag=f"xp{d}{rb}", name=f"xpd_{d}{rb}")
            nc.scalar.dma_start(xpd[:], xp_out[:])
            xp_dram[d][rb] = xpd

        # ---------------- recurrence ----------------
        c_prev = {d: None for d in DIRS}
        h_all = [None]                        # [128, 2, KT, 32] bf16 (both dirs)
        hst2 = [None] * S                     # fused per-step AG outputs (DRAM)
        h_last_f32 = {}
        c_last = {}

        def emit_dir_compute(d, t, agin2):
            """dir-step compute: fwd s=t, bwd s=S-1-t; h lands in agin2."""
            di = DIRS.index(d)
            s = t if d == "f" else S - 1 - t
            buf = t % 2
            gsl = gates_ps[d][:, 128 * buf:128 * (buf + 1)]

            xp_sb = sb.tile([32, G], F32, tag=f"xps{d}", bufs=3, name=f"xp_{d}{t}")
            rb, si = s // 4, s % 4
            nc.sync.dma_start(xp_sb[:], xp_dram[d][rb][32 * si:32 * (si + 1), :])

            # seed psum with xp (start=True clears the bank's has_written)
            for m in range(4):
                nc.tensor.matmul(
                    gsl[:, 32 * m:32 * (m + 1)],
                    xp_sb[:, 128 * m:128 * (m + 1)], i32f[:, :],
                    start=(m == 0), stop=(m == 3 and t == 0),
                )
            if t > 0:
                ha = h_all[0]
                for m in range(4):
                    for k in range(KT):
                        nc.tensor.matmul(
                            gsl[:, 32 * m:32 * (m + 1)],
                            whh[d][:, k, m, :], ha[:, k, di, :],
                            start=False,
                            stop=(m == 3 and k == KT - 1),
                        )

            # activations + cell update (i,g first so ig-mul starts early;
            # o last, overlapping the c-chain)
            act = {}
            for nm, m, fn in (("i", 0, AF.Sigmoid), ("g", 2, AF.Tanh),
                              ("f", 1, AF.Sigmoid), ("o", 3, AF.Sigmoid)):
                a = sb.tile([128, 32], F32, tag=f"a{nm}{d}", bufs=2, name=f"{nm}_{d}{t}")
                nc.scalar.activation(a[:], gsl[:, 32 * m:32 * (m + 1)], fn)
                act[nm] = a

            ig = sb.tile([128, 32], F32, tag=f"ig{d}", bufs=2, name=f"ig_{d}{t}")
            nc.vector.tensor_mul(ig[:], act["i"][:], act["g"][:])
            c_new = sb.tile([128, 32], F32, tag=f"c{d}", bufs=2, name=f"c_{d}{t}")
            if t == 0:
                nc.vector.tensor_copy(c_new[:], ig[:])
            else:
                fc = sb.tile([128, 32], F32, tag=f"fcm{d}", bufs=2, name=f"fcm_{d}{t}")
                nc.vector.tensor_mul(fc[:], act["f"][:], c_prev[d][:])
                nc.vector.tensor_add(c_new[:], fc[:], ig[:])
            c_prev[d] = c_new
            tc_sb = sb.tile([128, 32], F32, tag=f"tc{d}", bufs=2, name=f"tc_{d}{t}")
            nc.scalar.activation(tc_sb[:], c_new[:], AF.Tanh)
            h_bf = sb.tile([128, 32], BF16, tag=f"h{d}", bufs=2, name=f"h_{d}{t}")
            nc.vector.tensor_mul(h_bf[:], act["o"][:], tc_sb[:])

            if t == S - 1:
                h_f32 = sb.tile([128, 32], F32, tag=f"hf32{d}", name=f"h32_{d}{t}")
                nc.vector.tensor_mul(h_f32[:], act["o"][:], tc_sb[:])
                h_last_f32[d] = h_f32
                c_last[d] = c_new

            nc.sync.dma_start(agin2[128 * di:128 * (di + 1), :], h_bf[:])

        def emit_step(t):
            agin2 = dr.tile([256, 32], BF16, tag="agin", bufs=2, name=f"agin_{t}")
            emit_dir_compute("f", t, agin2)
            emit_dir_compute("b", t, agin2)
            hout = dr.tile([n_cores * 256, 32], BF16, tag=f"hst{t}", name=f"hst_{t}")
            nc.gpsimd.collective_compute(
                "AllGather", mybir.AluOpType.bypass,
                replica_groups=rg, ins=[agin2.opt()], outs=[hout.opt()],
            )
            hst2[t] = hout
            if t < S - 1:
                ha = sb.tile([128, KT, 2, 32], BF16, tag="hall", bufs=2,
                             name=f"hall_{t}")
                nc.sync.dma_start(
                    ha[:], hout.rearrange("(k two p) b -> p k two b", two=2, p=128))
                h_all[0] = ha

        def hst_view(d, s):
            """[128, KT, 32] view (p, k, b) of h_d[s] inside the fused AG out."""
            di = DIRS.index(d)
            t = s if d == "f" else S - 1 - s
            v = hst2[t].rearrange("(k two p) b -> p two k b", two=2, p=128)
            return v[:, di, :, :]

        # ---------------- fc_out pieces ----------------
        fc_lhs = {}     # rb -> lhs tile

        def emit_fc_lhs(rb):
            lhs = sb.tile([128, 2 * KT, 4, 32], BF16, tag="fclhs", bufs=4,
                          name=f"fclhs_{rb}")
            for di, d in enumerate(DIRS):
                for si in range(4):
                    nc.scalar.dma_start(
                        lhs[:, di * KT:(di + 1) * KT, si, :],
                        hst_view(d, 4 * rb + si),
                    )
            fc_lhs[rb] = lhs

        def emit_fc_piece(rb, half, khalf):
            """khalf 0: k 0..KT; khalf 1: k KT..2KT + bias + copies + DMA."""
            lhs = fc_lhs[rb]
            vbase = half * VHALF
            assert len(NCH) <= 4
            ks = range(0, KT) if khalf == 0 else range(KT, 2 * KT)
            for k in ks:
                for gi, (noff, nsz) in enumerate(NCH):
                    nc.tensor.matmul(
                        fc_ps[:, 512 * gi:512 * gi + nsz],
                        lhs[:, k, :, :],
                        wout_sb[0][:, k, noff:noff + nsz],
                        start=(k == 0), stop=False,
                    )
            if khalf == 0:
                return
            for gi, (noff, nsz) in enumerate(NCH):
                nc.tensor.matmul(
                    fc_ps[:, 512 * gi:512 * gi + nsz],
                    ones[:, :], bout_sb[:, vbase + noff:vbase + noff + nsz],
                    start=False, stop=True,
                )
            for gi, (noff, nsz) in enumerate(NCH):
                o_sb = sb.tile([128, 500], F32, tag="fco", bufs=4,
                               name=f"fco_{rb}_{half}_{gi}")
                if gi % 2 == 0:
                    nc.scalar.copy(o_sb[:, 0:nsz],
                                   fc_ps[:, 512 * gi:512 * gi + nsz])
                else:
                    nc.vector.tensor_copy(o_sb[:, 0:nsz],
                                          fc_ps[:, 512 * gi:512 * gi + nsz])
                # psum rows are (si, b) s-major; logits is [B, S, VSH]
                dst = logits_d[:, 4 * rb:4 * rb + 4,
                               vbase + noff:vbase + noff + nsz]
                dst = dst.transpose([1, 0, 2])   # (si, b, n) iteration
                nc.scalar.dma_start(dst, o_sb[:, 0:nsz])

        # ---------------- emission schedule ----------------
        emit_xp_rb("f", 0)
        emit_xp_rb("b", NRB - 1)
        emit_xp_rb("f", 1)
        emit_xp_rb("b", NRB - 2)

        fc_queue = []
        fc_ready = set()
        next_xp = 2

        for t in range(S):
            emit_step(t)
            if t % 4 == 1 and next_xp < NRB:
                emit_xp_rb("f", next_xp)
                emit_xp_rb("b", NRB - 1 - next_xp)
                next_xp += 1
            for rb in range(NRB):
                if rb not in fc_ready and t >= max(4 * rb + 3, S - 1 - 4 * rb):
                    fc_ready.add(rb)
                    for khalf in range(2):
                        fc_queue.append((rb, 0, khalf))
            if fc_queue and t >= 2:
                rb_, half_, khalf_ = fc_queue.pop(0)
                if khalf_ == 0:
                    emit_fc_lhs(rb_)
                emit_fc_piece(rb_, half_, khalf_)

        while fc_queue:
            rb_, half_, khalf_ = fc_queue.pop(0)
            if khalf_ == 0:
                emit_fc_lhs(rb_)
            emit_fc_piece(rb_, half_, khalf_)
        # second half of the vocab shard; prefetch lhs one rb ahead
        load_wout_half(1)
        emit_fc_lhs(0)
        for rb in range(NRB):
            if rb + 1 < NRB:
                emit_fc_lhs(rb + 1)
            for khalf in range(2):
                emit_fc_piece(rb, 1, khalf)

        # ---------------- final states ----------------
        st_sb = sb.tile([128, 4, 32], F32, tag="stin")
        for i, src in enumerate((h_last_f32["f"], c_last["f"],
                                 h_last_f32["b"], c_last["b"])):
            nc.vector.tensor_copy(st_sb[:, i, :], src[:])
        st_in = dr.tile([128, 128], F32, tag="stagin")
        nc.sync.dma_start(st_in[:], st_sb[:])
        st_all = dr.tile([n_cores * 128, 128], F32, tag="stall")
        nc.gpsimd.collective_compute(
            "AllGather", mybir.AluOpType.bypass,
            replica_groups=rg, ins=[st_in.opt()], outs=[st_all.opt()],
        )
        st_all_sb = sb.tile([128, n_cores, 128], F32, tag="stallsb")
        nc.sync.dma_start(st_all_sb[:],
                          st_all.rearrange("(c p) q -> p c q", p=128))
        stps = gates_ps["f"]     # gates banks are free after the last step
        for i, nm in enumerate(("hf", "cf", "hb", "cb")):
            out_sb = sb.tile([32, H], F32, tag="stout", bufs=2, name=f"st_{nm}")
            for c in range(n_cores):
                nc.tensor.transpose(
                    stps[0:32, 0:128],
                    st_all_sb[:, c, 32 * i:32 * (i + 1)],
                    i128f[:, :],
                )
                nc.vector.tensor_copy(out_sb[:, 128 * c:128 * (c + 1)],
                                      stps[0:32, 0:128])
            nc.sync.dma_start(st_out_d[nm][:, :], out_sb[:])

        sb.release()
        ps.release()
        dr.release()

    nc.compile()
    return nc


# ---------------------------------------------------------------------------
# host side
# ---------------------------------------------------------------------------

def make_in_maps(inputs, VSH=4000, n_cores=N_CORES):
    import ml_dtypes
    bf = ml_dtypes.bfloat16
    x = np.asarray(inputs["x"], np.int32)
    embed = np.asarray(inputs["embed"], np.float32)
    W = {d: {k: np.asarray(inputs[f"{k}_{d}"], np.float32)
             for k in ("W_ih", "b_ih", "W_hh", "b_hh")} for d in "fb"}
    W_out = np.asarray(inputs["W_out"], np.float32)
    b_out = np.asarray(inputs["b_out"], np.float32)
    Hh = W["f"]["W_ih"].shape[0]
    kt = Hh // 128
    hc = Hh // n_cores
    embed_bf = embed.astype(bf)
    in_maps = []
    for j in range(n_cores):
        cols = np.concatenate(
            [np.arange(g * Hh + j * hc, g * Hh + (j + 1) * hc) for g in range(4)])
        m = {
            "x": x,
            "embed_bf": embed_bf,
            "i32f": np.eye(32, dtype=np.float32),
            "i128b": np.eye(128, dtype=np.float32).astype(bf),
            "i128f": np.eye(128, dtype=np.float32),
            "onesb": np.ones((1, 128), np.float32).astype(bf),
        }
        g4 = 4 * hc
        for d in "fb":
            wihp = W[d]["W_ih"][:, cols].reshape(kt, 128, g4).transpose(1, 0, 2)
            m[f"wih_{d}"] = np.ascontiguousarray(wihp).astype(bf)
            whhp = (W[d]["W_hh"][:, cols]
                    .reshape(kt, 128, 4, hc).transpose(1, 0, 2, 3))
            m[f"whh_{d}"] = np.ascontiguousarray(whhp).astype(bf)
            m[f"bsum_{d}"] = ((W[d]["b_ih"] + W[d]["b_hh"])[cols][None, :]).astype(bf)
        vs = j * VSH
        wo = W_out[:, vs:vs + VSH].reshape(2 * kt, 128, VSH).transpose(1, 0, 2)
        m["wout"] = np.ascontiguousarray(wo).astype(bf)
        m["bout"] = b_out[vs:vs + VSH][None, :].astype(bf)
        in_maps.append(m)
    return in_maps


_NC_CACHE = {}


def _get_nc(key=(128, 4000, 32000)):
    if key not in _NC_CACHE:
        _NC_CACHE[key] = build_nc(S=key[0], VSH=key[1], VEMB=key[2])
    return _NC_CACHE[key]


def kernel(**inputs):
    S, VSH = 128, 4000
    nc = _get_nc((S, VSH, 32000))
    in_maps = make_in_maps(inputs, VSH=VSH)
    res = run_bass_kernel_spmd(nc, in_maps, core_ids=list(range(N_CORES)))
    outs = res.results
    logits = np.concatenate(
        [np.asarray(outs[j]["logits"], np.float32) for j in range(N_CORES)], axis=-1)
    st = outs[0]
    return (logits,
            (np.asarray(st["hf"], np.float32), np.asarray(st["cf"], np.float32),
             np.asarray(st["hb"], np.float32), np.asarray(st["cb"], np.float32)))
